# revision 1
# baseline (speedup 1.0000x reference)
"""Distributed Trainium2 kernel for gated causal self-attention.

Sharding: head-parallel across 8 cores (16 heads -> 2 heads/core, both
batches on every core).  Each core gets the full tokens, its 128-column
slice of Wq/Wk/Wv, its 2 columns of Wg and its 128-row slice of Wout, and
produces a full-shape [4096, 1024] partial of the output projection.  The
host sums the 8 partials (row-parallel Wout => sum reduction).

On-core pipeline (all matmuls bf16, fp32 accumulate):
  1. rmsnorm stats on natural-layout token tiles; transpose to xT via
     TensorE with diag(r) fused as the moving operand (rms scale free).
  2. qT/kT/vT/gates projections from xT (weights stationary).
  3. v natural layout via TensorE transpose of vT; ones column appended
     (softmax denominator comes out of the AV matmul for free).
  4. scores computed transposed (sT[j,i]) with the two heads row-packed
     (K=64 each) into one PE pass; exp on ScalarE (no max subtraction --
     logits are O(1) by construction); causal handled by skipping j>i
     column ranges + one 128x128 triangular mask on diagonal blocks.
  5. outT[h] accumulated over j-blocks in PSUM, row 64 = sum(exp) = l.
  6. gates*1/l broadcast across partitions with a tiny K=2 matmul;
     gated outT (both heads stacked = 128 rows) -> Wout partial -> DRAM.
"""

import functools
import os
import sys

import numpy as np

sys.path.insert(0, "/opt/trn_rl_repo")

import concourse.bass as bass
import concourse.mybir as mybir
import concourse.tile as tile
from concourse import bacc
from concourse.masks import make_identity, make_upper_triangular

P = 128
DIM = 1024
NB = 2
SEQ = 2048
NTOK = NB * SEQ  # 4096
KT = DIM // P  # 8 contraction tiles
DH = 64
SCALE = DH ** -0.5
N_CORES = 8
HEADS = 16
DIM_INNER = HEADS * DH

F32 = mybir.dt.float32
BF16 = mybir.dt.bfloat16
AF = mybir.ActivationFunctionType
ALU = mybir.AluOpType

NTT = NTOK // P      # 32 token tiles
NSL = NTOK // 512    # 8 projection slices
ISL_PER_B = SEQ // 512  # 4 i-slices per batch
JB_PER_B = SEQ // P     # 16 j-blocks per batch


def build_body(nc: bass.Bass, tc, io):
    tokens, wq, wk, wv, wg, wout, out = io

    # ---------------- persistent tensors ----------------
    consts = tc.alloc_tile_pool(name="consts", bufs=1)
    big = tc.alloc_tile_pool(name="big", bufs=1)

    ident_bf = consts.tile([P, P], BF16)
    make_identity(nc, ident_bf)
    trimask = consts.tile([P, P], BF16)
    make_upper_triangular(nc, trimask, val=1.0, diag=True)
    # sel_h[0, m] = 1 iff m in head h's 64-row block (for K=1 broadcast mm)
    sel0 = consts.tile([1, P], BF16)
    nc.vector.memset(sel0, 0.0)
    nc.vector.memset(sel0[0:1, 0:DH], 1.0)
    sel1 = consts.tile([1, P], BF16)
    nc.vector.memset(sel1, 0.0)
    nc.vector.memset(sel1[0:1, DH:P], 1.0)
    eps_sb = consts.tile([P, 1], F32)
    nc.vector.memset(eps_sb, float(np.finfo(np.float32).eps))

    # weights (bf16)
    wq_bf = consts.tile([P, KT, P], BF16)
    wk_bf = consts.tile([P, KT, P], BF16)
    wv_bf = consts.tile([P, KT, P], BF16)
    wg_bf = consts.tile([P, KT, 2], BF16)
    wout_bf = consts.tile([P, DIM], BF16)

    with tc.tile_pool(name="wstage", bufs=5) as wstage:
        for src, dst, cols in ((wq, wq_bf, P), (wk, wk_bf, P), (wv, wv_bf, P),
                               (wg, wg_bf, 2)):
            st = wstage.tile([P, KT, cols], F32, tag="wst")
            nc.sync.dma_start(out=st, in_=src.rearrange("(o p) m -> p o m", p=P))
            nc.vector.tensor_copy(out=dst, in_=st)
        st = wstage.tile([P, DIM], F32, tag="wst")
        nc.sync.dma_start(out=st, in_=wout[:, :])
        nc.vector.tensor_copy(out=wout_bf, in_=st)

    xT = big.tile([P, KT, NTOK], BF16)       # 8 MiB
    qT = big.tile([P, NTOK], BF16)           # 1 MiB
    kT = big.tile([P, NTOK], BF16)
    vT = big.tile([P, NTOK], BF16)
    v_nat = big.tile([P, NTT, 130], BF16)    # [j, jb, h*65+c]; col 64/129 = ones

    nc.vector.memset(v_nat[:, :, DH], 1.0)
    nc.vector.memset(v_nat[:, :, DH + 65], 1.0)

    # ------- fused per-slice pipeline: norm+transpose -> proj -> attn -------
    with tc.tile_pool(name="toks", bufs=3) as toks, \
         tc.tile_pool(name="smallB", bufs=8) as small, \
         tc.tile_pool(name="ppool", bufs=6) as ppool, \
         tc.tile_pool(name="smE", bufs=3) as smE, \
         tc.tile_pool(name="opool", bufs=2) as opool, \
         tc.tile_pool(name="mmps", bufs=3, space="PSUM") as mmps, \
         tc.tile_pool(name="sps", bufs=3, space="PSUM") as sps, \
         tc.tile_pool(name="ops", bufs=1, space="PSUM") as ops:
        for b in range(NB):
            for i0 in range(ISL_PER_B):
                it = b * ISL_PER_B + i0
                isl0 = it * 512
                sl = slice(isl0, isl0 + 512)
                # --- B: token load, rmsnorm stats, transpose to xT slice ---
                tokq = toks.tile([P, 4, DIM], BF16, tag="tok")
                nc.sync.dma_start(
                    out=tokq,
                    in_=tokens[isl0:isl0 + 512, :].rearrange(
                        "(o p) d -> p o d", p=P))
                for o in range(4):
                    tok = tokq[:, o, :]
                    tt = it * 4 + o
                    sq = toks.tile([P, DIM], BF16, tag="sq")
                    ssq = small.tile([P, 1], F32, tag="ssq")
                    nc.scalar.activation(out=sq, in_=tok, func=AF.Square,
                                         accum_out=ssq)
                    r = small.tile([P, 1], F32, tag="r")
                    nc.scalar.activation(out=r, in_=ssq, func=AF.Ln,
                                         bias=eps_sb, scale=1.0 / DIM)
                    nc.scalar.activation(out=r, in_=r, func=AF.Exp,
                                         scale=-0.5)
                    diag = small.tile([P, P], BF16, tag="diag")
                    nc.gpsimd.tensor_scalar_mul(diag, ident_bf, r)
                    for kh in range(2):
                        pt = mmps.tile([P, 4, P], F32, tag="mm")
                        for ki in range(4):
                            kb = kh * 4 + ki
                            nc.tensor.matmul(pt[:, ki, :],
                                             lhsT=tok[:, kb * P:(kb + 1) * P],
                                             rhs=diag, start=True, stop=True)
                        dst = xT[:, kh * 4:(kh + 1) * 4, tt * P:(tt + 1) * P]
                        nc.vector.tensor_copy(out=dst, in_=pt)
                # --- C: projections for this slice ---
                for wt, dstT in ((wq_bf, qT), (wk_bf, kT), (wv_bf, vT)):
                    ps = mmps.tile([P, 512], F32, tag="mm")
                    for kb in range(KT):
                        nc.tensor.matmul(ps, lhsT=wt[:, kb, :],
                                         rhs=xT[:, kb, sl],
                                         start=(kb == 0), stop=(kb == KT - 1))
                    nc.vector.tensor_copy(out=dstT[:, sl], in_=ps)
                pg = mmps.tile([2, 512], F32, tag="mm")
                for kb in range(KT):
                    nc.tensor.matmul(pg, lhsT=wg_bf[:, kb, :],
                                     rhs=xT[:, kb, sl],
                                     start=(kb == 0), stop=(kb == KT - 1))
                g_sl = smE.tile([2, 512], F32, tag="gsl")
                nc.scalar.activation(out=g_sl, in_=pg, func=AF.Exp,
                                     scale=-1.0)
                nc.vector.tensor_scalar_add(g_sl, g_sl, 1.0)
                nc.vector.reciprocal(out=g_sl, in_=g_sl)
                g1_sl = smE.tile([1, 512], F32, tag="g1sl")
                nc.sync.dma_start(out=g1_sl, in_=g_sl[1:2, :])
                # --- D: v natural for this slice's j-blocks ---
                for o in range(4):
                    jb = it * 4 + o
                    pv = mmps.tile([P, P], F32, tag="mm")
                    nc.tensor.matmul(pv, lhsT=vT[:, jb * P:(jb + 1) * P],
                                     rhs=ident_bf, start=True, stop=True)
                    nc.vector.tensor_copy(out=v_nat[:, jb, 0:DH],
                                           in_=pv[:, 0:DH])
                    nc.vector.tensor_copy(out=v_nat[:, jb, 65:65 + DH],
                                           in_=pv[:, DH:P])
                # --- E: attention for islice (b, i0) ---
                po = [ops.tile([P, 512], F32, tag=f"o{h}", name=f"o{h}")
                      for h in (0, 1)]
                njb = 4 * i0 + 4
                for j in range(njb):
                    jb = b * JB_PER_B + j
                    off = max(0, (j - 4 * i0) * P)
                    n = 512 - off
                    first, last = (j == 0), (j == njb - 1)
                    ps_s = []
                    for h in (0, 1):
                        hp = slice(h * DH, (h + 1) * DH)
                        ss = sps.tile([P, 512], F32, tag="s")
                        nc.tensor.matmul(
                            ss[:, :n], lhsT=kT[hp, jb * P:(jb + 1) * P],
                            rhs=qT[hp, isl0 + off:isl0 + 512],
                            start=True, stop=True)
                        ps_s.append(ss)
                    for h in (0, 1):
                        pexp = ppool.tile([P, 512], BF16, tag="p")
                        nc.scalar.activation(out=pexp[:, :n], in_=ps_s[h][:, :n],
                                             func=AF.Exp, scale=SCALE)
                        if j >= 4 * i0:  # diagonal block: mask j > i
                            nc.gpsimd.tensor_tensor(
                                pexp[:, 0:P], pexp[:, 0:P], trimask, ALU.mult)
                        nc.tensor.matmul(
                            po[h][:65, off:512],
                            lhsT=v_nat[:, jb, h * 65:(h + 1) * 65],
                            rhs=pexp[:, :n], start=first, stop=last)
                # gates * 1/l, broadcast to 64 partitions per head via K=1 mms
                gsrc = (g_sl[0:1, :], g1_sl[0:1, :])
                glb_bf = []
                for h in (0, 1):
                    gl = smE.tile([1, 512], F32, tag=f"gl{h}", name=f"gl{h}")
                    nc.vector.reciprocal(out=gl, in_=po[h][DH:DH + 1, :])
                    nc.vector.tensor_tensor(gl, gl, gsrc[h], ALU.mult)
                    glb = smE.tile([1, 512], BF16, tag=f"glb{h}", name=f"glb{h}")
                    nc.vector.tensor_copy(out=glb, in_=gl)
                    glb_bf.append(glb)
                pb = sps.tile([P, 512], F32, tag="s")
                nc.tensor.matmul(pb, lhsT=sel0, rhs=glb_bf[0], start=True,
                                 stop=False)
                nc.tensor.matmul(pb, lhsT=sel1, rhs=glb_bf[1], start=False,
                                 stop=True)
                pbs = smE.tile([P, 512], F32, tag="pbs")
                nc.scalar.copy(out=pbs, in_=pb)
                goT = opool.tile([P, 512], BF16, tag="goT")
                for h in (0, 1):
                    nc.vector.tensor_tensor(
                        goT[h * DH:(h + 1) * DH, :], po[h][0:DH, :],
                        pbs[h * DH:(h + 1) * DH, :], ALU.mult)
                # Wout: partial[t, :] += goT.T @ wout  (bf16 partial out)
                osb = opool.tile([P, 4, DIM], BF16, tag="osb")
                for lt in range(4):
                    for c in (0, 1):
                        pw = sps.tile([P, 512], F32, tag="s")
                        nc.tensor.matmul(pw, lhsT=goT[:, lt * P:(lt + 1) * P],
                                         rhs=wout_bf[:, c * 512:(c + 1) * 512],
                                         start=True, stop=True)
                        if lt % 2 == 1 and c == 1:
                            nc.scalar.copy(
                                out=osb[:, lt, c * 512:(c + 1) * 512], in_=pw)
                        else:
                            nc.vector.tensor_copy(
                                out=osb[:, lt, c * 512:(c + 1) * 512], in_=pw)
                nc.sync.dma_start(
                    out=out[isl0:isl0 + 512, :].rearrange(
                        "(o p) d -> p o d", p=P),
                    in_=osb)

    big.release()
    consts.release()


def build_graph() -> bass.Bass:
    nc = bacc.Bacc(None, target_bir_lowering=False, debug=False)
    tokens = nc.declare_dram_parameter("tokens", [NTOK, DIM], BF16, isOutput=False)
    wq = nc.declare_dram_parameter("wq", [DIM, P], F32, isOutput=False)
    wk = nc.declare_dram_parameter("wk", [DIM, P], F32, isOutput=False)
    wv = nc.declare_dram_parameter("wv", [DIM, P], F32, isOutput=False)
    wg = nc.declare_dram_parameter("wg", [DIM, 2], F32, isOutput=False)
    wout = nc.declare_dram_parameter("wout", [P, DIM], F32, isOutput=False)
    out = nc.declare_dram_parameter("out", [NTOK, DIM], BF16, isOutput=True)
    with tile.TileContext(nc) as tc:
        build_body(nc, tc, (tokens, wq, wk, wv, wg, wout, out))
    nc.compile()
    return nc


@functools.lru_cache(maxsize=1)
def _graph():
    return build_graph()


LAST_RESULT = None


def kernel(tokens, norm_w, Wq, Wkv, Wout, Wg):
    global LAST_RESULT
    from concourse.bass_utils import run_bass_kernel_spmd

    import ml_dtypes
    tokens = np.ascontiguousarray(
        np.asarray(tokens, np.float32).reshape(NTOK, DIM).astype(ml_dtypes.bfloat16))
    norm_w = np.ascontiguousarray(np.asarray(norm_w, np.float32))
    Wq = np.asarray(Wq, np.float32)
    Wkv = np.asarray(Wkv, np.float32)
    Wout = np.asarray(Wout, np.float32)
    Wg = np.asarray(Wg, np.float32)
    nw = norm_w[:, None]
    Wq = Wq * nw
    Wkv = Wkv * nw
    Wg = Wg * nw
    Wk_all = Wkv[:, :DIM_INNER]
    Wv_all = Wkv[:, DIM_INNER:]

    in_maps = []
    for i in range(N_CORES):
        cs = slice(i * P, (i + 1) * P)
        in_maps.append({
            "tokens": tokens,
            "wq": np.ascontiguousarray(Wq[:, cs]),
            "wk": np.ascontiguousarray(Wk_all[:, cs]),
            "wv": np.ascontiguousarray(Wv_all[:, cs]),
            "wg": np.ascontiguousarray(Wg[:, 2 * i:2 * i + 2]),
            "wout": np.ascontiguousarray(Wout[cs, :]),
        })

    res = run_bass_kernel_spmd(
        _graph(), in_maps, core_ids=list(range(N_CORES)),
        trace=bool(int(os.environ.get("KERNEL_TRACE", "0"))))
    LAST_RESULT = res
    total = np.zeros((NTOK, DIM), np.float32)
    for r in res.results:
        total += np.asarray(r["out"], np.float32)
    return total.reshape(NB, SEQ, DIM)


if __name__ == "__main__":
    # smoke test with random data
    rng = np.random.default_rng(0)
    toks = rng.standard_normal((NB, SEQ, DIM), dtype=np.float32)
    o = kernel(
        tokens=toks,
        norm_w=np.ones(DIM, np.float32),
        Wq=(rng.standard_normal((DIM, DIM_INNER)) * 0.02).astype(np.float32),
        Wkv=(rng.standard_normal((DIM, 2 * DIM_INNER)) * 0.02).astype(np.float32),
        Wout=(rng.standard_normal((DIM_INNER, DIM)) * 0.02).astype(np.float32),
        Wg=(rng.standard_normal((DIM, HEADS)) * 0.02).astype(np.float32),
    )
    print("out", o.shape, o.dtype, float(np.abs(o).mean()))

